# revision 2
# baseline (speedup 1.0000x reference)
"""BitLinear (RMSNorm + ternary linear) Trainium2 kernel, 8-way SPMD,
fp8 e4m3 DoubleRow TensorE path.

Math (identical to the reference up to quantization):
    rms   = sqrt(mean(x^2, axis=-1) + 1e-6)
    y     = (x @ w_q.T) / rms * gamma        (norm_weight == 1 fast path)

Precision scheme: x is quantized to fp8 e4m3 (hi); for k < NB*128 the
residual lo = e4m3(x - hi) rides the second DoubleRow lane. Ternary
weights are exact in fp8. Each DoubleRow matmul contracts 256 values
(2 fp8 lanes/cell, 2 MACs/cycle; HW-verified 216ns per
[256k x 128o x 512t] instruction), so the GEMM costs NS=23 slot-matmuls
per output tile instead of 32 bf16 matmuls: 0.72x the bf16 PE time at
~1.80% rel error (exact-input simulated + HW-verified; gate 2e-2).

Slot layout (identical for every output group):
  slots 0..NB-1   type-B: k in [128j, 128j+128), lane0=hi, lane1=lo,
                  weights stored once, lane dim stride-0 broadcast.
  slots NB..NS-1  type-A: k in [NB*128+256j, ...+256), two k-tiles.

Device pipeline per core (1024 tokens, data-parallel over 8 cores):
  - 32 output groups of 128; per og, 23 slot-matmuls x 2 token halves
    accumulate in 2 PSUM banks; og-major order so banks release (DVE
    bf16 copy) and epilogues stagger continuously; 8 bank tags give a
    4-og in-flight window. Stationary weight tiles (merged wb+wa, 4KB
    per partition per og) stream on the sync queue as 1MB og-pair DMAs.
  - Panel 0 (og 0-3) issues matmuls in a DMA-arrival-aware greedy order
    while xt (6MB) streams on scalar/gpsimd; afterwards xt is resident.
  - rstd never gates bank release: per-og epilogue is DVE copy ->
    ACT gamma (per-partition scale); the DVE rstd row-mul + out DMA are
    deferred for og 0-11 (by then the stats pipeline - ACT Square/Sqrt,
    DVE reciprocal on a t-major fp8 x copy, DRAM-bounced row broadcast -
    has certainly drained) and inline afterwards.
Output leaves transposed ([D_OUT, TOK] per core); host transposes back.
"""

import numpy as np
import ml_dtypes

import concourse.bass as bass
import concourse.tile as tile
from concourse import bacc, mybir
from concourse.bass_utils import run_bass_kernel_spmd

N_CORES = 8
B, S_SEQ, D_IN = 2, 4096, 4096
D_OUT = 4096
TOK_TOTAL = B * S_SEQ          # 8192
TOK = TOK_TOTAL // N_CORES     # 1024 tokens per core
P = 128
NB = 12                        # type-B slots (hi/lo, k < 1536)
NA = (D_IN - NB * P) // 256    # 10 type-A slots (hi/hi)
NS = NB + NA                   # 22 k slots
NOG = D_OUT // P               # 32 output groups of 128
TH = 2                         # token halves
TN = TOK // TH                 # 512 tokens per matmul
WROW = NB * P + NA * 2 * P     # 4096 weight bytes/partition/og
EPS_NORM = 1e-6
N_WARM = 14
DEFER_OG = 12                  # ogs whose rstd-mul + out DMA are deferred

F32 = mybir.dt.float32
BF16 = mybir.dt.bfloat16
FP8 = mybir.dt.float8e4
DR = mybir.MatmulPerfMode.DoubleRow
E4 = ml_dtypes.float8_e4m3

LAST_RESULTS = None

# startup DMA plans
SCALAR_PLAN = ([("B", (s,)) for s in range(4)] +
               [("B", (4, 5)), ("B", (6, 7)), ("B", (8, 9)), ("B", (10, 11))])
GPSIMD_PLAN = ([("gamma", None)] + [("A", (0,)), ("A", (1,))] +
               [("A", (2, 3)), ("A", (4, 5)), ("A", (6, 7)), ("A", (8, 9))])
SYNC_PLAN = [("W", (g,)) for g in range(4)]  # singles; pairs follow per-panel


def _pan0_schedule():
    """Greedy og0-3 matmul order under modeled DMA arrival times (us,
    ~65GB/s per queue early, 2.5us queue start). Returns (oi, slot) and
    ("fill",) entries; correctness is enforced by Tile semaphores - this
    only minimizes issue-order stalls, and filler matmuls absorb the
    DMA-gated idle so HAM never re-throttles."""
    MBUS = 1e3 / 65e3  # us per KB

    xt_av = [0.0] * NS
    w_av = [0.0] * 4
    t = 2.5
    for kind, payload in SCALAR_PLAN:
        t += 262 * len(payload) * MBUS
        for s in payload:
            xt_av[s] = t
    t = 2.5
    for kind, payload in GPSIMD_PLAN:
        if kind == "gamma":
            t += 16 * MBUS
            continue
        t += 262 * len(payload) * MBUS
        for j in payload:
            xt_av[NB + j] = t
    t = 2.5
    for kind, payload in SYNC_PLAN:
        t += 524 * MBUS
        w_av[payload[0]] = t

    nxt = [0] * 4
    seq = []
    t = 8.0
    n_fill = 0
    while any(n < NS for n in nxt):
        progressed = False
        for oi in range(4):
            if nxt[oi] >= NS or t < w_av[oi] or t < xt_av[nxt[oi]]:
                continue
            seq.append((oi, nxt[oi]))
            nxt[oi] += 1
            t += 2 * 0.216
            progressed = True
        if not progressed:
            pend = [xt_av[nxt[oi]] for oi in range(4) if nxt[oi] < NS]
            pend += [w_av[oi] for oi in range(4) if nxt[oi] == 0]
            fut = [p for p in pend if p > t]
            tn = max(t + 0.216, min(fut)) if fut else t + 0.216
            # fillers keep the PE busy only before og3 starts (its PSUM
            # bank tag doubles as the filler bank)
            if nxt[3] == 0 and n_fill < 40:
                k = min(int((tn - t) / 0.216) + 1, 40 - n_fill)
                seq += [("fill", None)] * k
                n_fill += k
            t = tn
    return seq


def build_nc(do_compile=True):
    nc = bacc.Bacc(
        "TRN2",
        target_bir_lowering=False,
        debug=False,
        enable_asserts=True,
        num_devices=N_CORES,
    )

    xt_ext = nc.declare_dram_parameter("xt", [NS, P, 2, TOK], FP8, isOutput=False)
    xs_ext = nc.declare_dram_parameter("xs", [TOK, D_IN], FP8, isOutput=False)
    # merged weights: w[g, i, :] = [wb rows | wa rows], 4KB/partition/og
    w_ext = nc.declare_dram_parameter("w", [NOG, P, WROW], FP8, isOutput=False)
    gamma_ext = nc.declare_dram_parameter("gamma", [D_OUT], F32, isOutput=False)
    rstd_ext = nc.declare_dram_parameter("rstd", [TOK], F32, isOutput=True)
    out_ext = nc.declare_dram_parameter("out", [D_OUT, TOK], BF16, isOutput=True)

    def row_bcast(ap_1d):
        return bass.AP(
            tensor=ap_1d.tensor, offset=ap_1d.offset,
            ap=[[0, P]] + list(ap_1d.ap),
        )

    with tile.TileContext(nc) as tc:
        with (
            tc.tile_pool(name="singles", bufs=1) as singles,
            tc.tile_pool(name="xspool", bufs=3) as xspool,
            tc.tile_pool(name="stats", bufs=2) as stats,
            tc.tile_pool(name="wpool", bufs=7) as wpool,
            tc.tile_pool(name="opool", bufs=18) as opool,
            tc.tile_pool(name="psum", bufs=1, space="PSUM") as psum,
        ):
            # ---- PE warmup first: issues during the DMA preamble ----
            warm_l = singles.tile([P, P], BF16)
            warm_r = singles.tile([P, TN], BF16)
            nc.vector.memset(warm_l, 0.0)
            nc.vector.memset(warm_r, 0.0)
            warm_ps = psum.tile([P, TN], F32, tag="ps0", name="warm_ps")
            for i in range(N_WARM):
                nc.tensor.matmul(
                    warm_ps, lhsT=warm_l, rhs=warm_r,
                    start=(i == 0), stop=(i == N_WARM - 1),
                )

            # ---- constants ----
            gamma_sb = singles.tile([P, NOG], F32)
            eps_sb = singles.tile([P, 1], F32)
            nc.vector.memset(eps_sb, EPS_NORM)
            rstd_all = singles.tile([P, 8], F32)
            rstd_bc = singles.tile([P, TOK], F32)

            xt_view = [None] * NS   # slot -> (tile, idx)
            w_view = {}             # og -> (tile, idx)

            def load_xt(slots, eng):
                n = len(slots)
                t = singles.tile(
                    [P, n, 2, TOK], FP8, tag=f"xt{slots[0]}",
                    name=f"xt_{slots[0]}",
                )
                eng.dma_start(
                    out=t,
                    in_=xt_ext[slots[0] : slots[0] + n].rearrange(
                        "s p two t -> p s two t"
                    ),
                )
                for j, s in enumerate(slots):
                    xt_view[s] = (t, j)

            def load_w(gs):
                n = len(gs)
                t = wpool.tile([P, n, WROW], FP8, tag="w", name=f"w_{gs[0]}")
                nc.sync.dma_start(
                    out=t,
                    in_=w_ext[gs[0] : gs[0] + n].rearrange("g p r -> p g r"),
                )
                for j, g in enumerate(gs):
                    w_view[g] = (t, j)

            # ---- startup DMAs ----
            for kind, payload in SYNC_PLAN:
                load_w(payload)
            load_w((4, 5))
            load_w((6, 7))
            for kind, payload in SCALAR_PLAN:
                load_xt(payload, nc.scalar)
            for kind, payload in GPSIMD_PLAN:
                if kind == "gamma":
                    nc.gpsimd.dma_start(
                        out=gamma_sb,
                        in_=gamma_ext.ap().rearrange("(g p) -> p g", p=P),
                    )
                else:
                    load_xt(tuple(NB + j for j in payload), nc.gpsimd)

            xs_tiles = [None] * 8
            for st in range(4, 8):
                t = xspool.tile([P, D_IN], FP8, tag="xs", name=f"xs_{st}")
                nc.gpsimd.dma_start(out=t, in_=xs_ext[st * P : (st + 1) * P, :])
                xs_tiles[st] = t
            for st in range(0, 4):
                t = xspool.tile([P, D_IN], FP8, tag="xs", name=f"xs_{st}")
                nc.scalar.dma_start(out=t, in_=xs_ext[st * P : (st + 1) * P, :])
                xs_tiles[st] = t

            # ---- norm statistics for one token half (4 strips) ----
            def stats_half(h):
                for st in range(h * 4, h * 4 + 4):
                    sq_dummy = stats.tile(
                        [P, D_IN], BF16, tag="sq", name=f"sq_{st}"
                    )
                    sumsq = stats.tile([P, 1], F32, tag="ss", name=f"ss_{st}")
                    nc.scalar.activation(
                        out=sq_dummy,
                        in_=xs_tiles[st],
                        func=mybir.ActivationFunctionType.Square,
                        accum_out=sumsq,
                    )
                    rcol = rstd_all[:, st : st + 1]
                    nc.scalar.activation(
                        out=rcol,
                        in_=sumsq,
                        func=mybir.ActivationFunctionType.Sqrt,
                        bias=eps_sb,
                        scale=1.0 / D_IN,
                    )
                    nc.vector.reciprocal(out=rcol, in_=rcol)
                nc.scalar.dma_start(
                    out=rstd_ext[h * TN : (h + 1) * TN].rearrange(
                        "(s p) -> p s", p=P
                    ),
                    in_=rstd_all[:, h * 4 : h * 4 + 4],
                )
                nc.scalar.dma_start(
                    out=rstd_bc[:, h * TN : (h + 1) * TN],
                    in_=row_bcast(rstd_ext[h * TN : (h + 1) * TN]),
                )

            def slot_lhsT(g, s):
                wt, gi = w_view[g]
                if s < NB:
                    return (
                        wt[:, gi, s * P : (s + 1) * P]
                        .unsqueeze(1)
                        .broadcast_to([P, 2, P])
                    )
                off = NB * P + (s - NB) * 2 * P
                return wt[:, gi, off : off + 2 * P].rearrange(
                    "p (two m) -> p two m", two=2
                )

            def mm(ps_pair, g, s, first, last):
                lhsT = slot_lhsT(g, s)
                for th in range(TH):
                    xt_t, j = xt_view[s]
                    nc.tensor.matmul(
                        ps_pair[th],
                        lhsT=lhsT,
                        rhs=xt_t[:, j, :, th * TN : (th + 1) * TN],
                        start=first,
                        stop=last,
                        perf_mode=DR,
                    )

            deferred = []  # (g, o_tile) awaiting rstd mul + out DMA

            def og_epilogue(g, ps_pair):
                o_tile = opool.tile([P, TOK], BF16, tag="o", name=f"o_{g}")
                for th in range(TH):
                    nc.vector.tensor_copy(
                        o_tile[:, th * TN : (th + 1) * TN], ps_pair[th]
                    )
                nc.scalar.activation(
                    out=o_tile,
                    in_=o_tile,
                    func=mybir.ActivationFunctionType.Copy,
                    scale=gamma_sb[:, g : g + 1],
                )
                deferred.append((g, o_tile))
                if g == DEFER_OG - 1 or g >= DEFER_OG:
                    while deferred:
                        dg, dt = deferred.pop(0)
                        nc.vector.tensor_mul(dt, dt, rstd_bc)
                        eng = nc.gpsimd if dg % 2 == 0 else nc.scalar
                        eng.dma_start(
                            out=out_ext[dg * P : (dg + 1) * P, :], in_=dt
                        )

            def alloc_ps(g):
                b = (g % 4) * TH
                return [
                    psum.tile([P, TN], F32, tag=f"ps{b + th}", name=f"ps_{g}_{th}")
                    for th in range(TH)
                ]

            # ---- og 0-3: arrival-aware greedy order; filler matmuls
            # (into og3's not-yet-started bank) bridge DMA-gated idle ----
            fill_ps = psum.tile([P, TN], F32, tag="ps7", name="fill_ps")
            ps_of = {oi: alloc_ps(oi) for oi in range(4)}
            prog = [0] * 4
            for oi, s in _pan0_schedule():
                if oi == "fill":
                    nc.tensor.matmul(
                        fill_ps, lhsT=warm_l, rhs=warm_r,
                        start=True, stop=True,
                    )
                    continue
                mm(ps_of[oi], oi, s, prog[oi] == 0, prog[oi] == NS - 1)
                prog[oi] += 1
                if prog[oi] == NS:
                    og_epilogue(oi, ps_of[oi])
                    if oi == 3:
                        stats_half(1)
                        stats_half(0)

            # ---- og 4-31: og-major, everything resident/prefetched ----
            for g in range(4, NOG):
                if g % 2 == 0:
                    pre = g + 4
                    if pre < NOG and pre not in w_view:
                        load_w((pre, pre + 1))
                ps_pair = alloc_ps(g)
                for s in range(NS):
                    mm(ps_pair, g, s, s == 0, s == NS - 1)
                og_epilogue(g, ps_pair)

    if do_compile:
        nc.compile()
    return nc


_NC_CACHE = {}


def _pack_weights(w_q):
    w8 = np.asarray(w_q, dtype=np.float32).astype(E4)
    # wb[g, i, j, m] = w_q[128g + m, 128j + i]           (k = 128j + i)
    wB = w8[:, : NB * P].reshape(NOG, P, NB, P).transpose(0, 3, 2, 1)
    # wa[g, i, j, l, m] = w_q[128g + m, NB*128 + 256j + 128l + i]
    wA = w8[:, NB * P :].reshape(NOG, P, NA, 2, P).transpose(0, 4, 2, 3, 1)
    wflat = np.concatenate(
        [wB.reshape(NOG, P, NB * P), wA.reshape(NOG, P, NA * 2 * P)], axis=-1
    )
    return np.ascontiguousarray(wflat)


def _pack_x(xc_hi, xc_lo):
    xt = np.empty((NS, P, 2, TOK), dtype=E4)
    hiB = xc_hi[:, : NB * P].reshape(TOK, NB, P)
    loB = xc_lo.reshape(TOK, NB, P)
    xt[:NB, :, 0, :] = hiB.transpose(1, 2, 0)
    xt[:NB, :, 1, :] = loB.transpose(1, 2, 0)
    hiA = xc_hi[:, NB * P :].reshape(TOK, NA, 2, P)
    xt[NB:, :, :, :] = hiA.transpose(1, 3, 2, 0)
    return xt


def kernel(x, norm_weight, w_q, gamma):
    global LAST_RESULTS
    x32 = np.ascontiguousarray(np.asarray(x, dtype=np.float32)).reshape(
        TOK_TOTAL, D_IN
    )
    nw = np.asarray(norm_weight, dtype=np.float32)
    g32 = np.ascontiguousarray(np.asarray(gamma, dtype=np.float32))

    hi_raw = x32.astype(E4)       # stats source (pre norm_weight)
    if np.all(nw == 1.0):
        hi = hi_raw
    else:
        x32 = x32 * nw
        hi = x32.astype(E4)
    lo = (x32[:, : NB * P] - hi[:, : NB * P].astype(np.float32)).astype(E4)

    wflat = _pack_weights(w_q)

    if "nc" not in _NC_CACHE:
        _NC_CACHE["nc"] = build_nc()
    nc = _NC_CACHE["nc"]

    in_maps = []
    for c in range(N_CORES):
        sl = slice(c * TOK, (c + 1) * TOK)
        in_maps.append(
            {
                "xt": _pack_x(hi[sl], lo[sl]),
                "xs": np.ascontiguousarray(hi_raw[sl]),
                "w": wflat,
                "gamma": g32,
            }
        )
    res = run_bass_kernel_spmd(nc, in_maps, core_ids=list(range(N_CORES)))
    LAST_RESULTS = res
    out = np.empty((TOK_TOTAL, D_OUT), dtype=np.float32)
    for c in range(N_CORES):
        ot = np.asarray(res.results[c]["out"])  # [D_OUT, TOK] bf16
        out[c * TOK : (c + 1) * TOK] = ot.T
    return out.reshape(B, S_SEQ, D_OUT)


# revision 3
# speedup vs baseline: 1.1792x; 1.1792x over previous
"""BitLinear (RMSNorm + ternary linear) Trainium2 kernel, 8-way SPMD,
fp8 e4m3 DoubleRow TensorE path.

Math (identical to the reference up to quantization):
    rms   = sqrt(mean(x^2, axis=-1) + 1e-6)
    y     = (x @ w_q.T) / rms * gamma        (norm_weight == 1 fast path)

Precision scheme: x is quantized to fp8 e4m3 (hi); for k < NB*128 the
residual lo = e4m3(x - hi) rides the second DoubleRow lane. Ternary
weights are exact in fp8. Each DoubleRow matmul contracts 256 values
(2 fp8 lanes/cell, 2 MACs/cycle; HW-verified 216ns per
[256k x 128o x 512t] instruction), so the GEMM costs NS=23 slot-matmuls
per output tile instead of 32 bf16 matmuls: 0.72x the bf16 PE time at
~1.80% rel error (exact-input simulated + HW-verified; gate 2e-2).

Slot layout (identical for every output group):
  slots 0..NB-1   type-B: k in [128j, 128j+128), lane0=hi, lane1=lo,
                  weights stored once, lane dim stride-0 broadcast.
  slots NB..NS-1  type-A: k in [NB*128+256j, ...+256), two k-tiles.

Device pipeline per core (1024 tokens, data-parallel over 8 cores):
  - 32 output groups of 128; per og, 23 slot-matmuls x 2 token halves
    accumulate in 2 PSUM banks; og-major order so banks release (DVE
    bf16 copy) and epilogues stagger continuously; 8 bank tags give a
    4-og in-flight window. Stationary weight tiles (merged wb+wa, 4KB
    per partition per og) stream on the sync queue as 1MB og-pair DMAs.
  - Panel 0 (og 0-3) issues matmuls in a DMA-arrival-aware greedy order
    while xt (6MB) streams on scalar/gpsimd; afterwards xt is resident.
  - rstd never gates bank release: per-og epilogue is DVE copy ->
    ACT gamma (per-partition scale); the DVE rstd row-mul + out DMA are
    deferred for og 0-11 (by then the stats pipeline - ACT Square/Sqrt,
    DVE reciprocal on a t-major fp8 x copy, DRAM-bounced row broadcast -
    has certainly drained) and inline afterwards.
Output leaves transposed ([D_OUT, TOK] per core); host transposes back.
"""

import numpy as np
import ml_dtypes

import concourse.bass as bass
import concourse.tile as tile
from concourse import bacc, mybir
from concourse.bass_utils import run_bass_kernel_spmd

N_CORES = 8
B, S_SEQ, D_IN = 2, 4096, 4096
D_OUT = 4096
TOK_TOTAL = B * S_SEQ          # 8192
TOK = TOK_TOTAL // N_CORES     # 1024 tokens per core
P = 128
NB = 12                        # type-B slots (hi/lo, k < 1536)
NA = (D_IN - NB * P) // 256    # 10 type-A slots (hi/hi)
NS = NB + NA                   # 22 k slots
NOG = D_OUT // P               # 32 output groups of 128
TH = 2                         # token halves
TN = TOK // TH                 # 512 tokens per matmul
WROW = NB * P + NA * 2 * P     # 4096 weight bytes/partition/og
EPS_NORM = 1e-6
N_WARM = 14
DEFER_OG = 12                  # ogs whose rstd-mul + out DMA are deferred

F32 = mybir.dt.float32
BF16 = mybir.dt.bfloat16
FP8 = mybir.dt.float8e4
DR = mybir.MatmulPerfMode.DoubleRow
E4 = ml_dtypes.float8_e4m3

LAST_RESULTS = None

# startup DMA plans (B8-11 ride the fast sync queue between weight loads)
SCALAR_PLAN = ([("B", (s,)) for s in range(4)] +
               [("B", (4, 5)), ("B", (6, 7))])
GPSIMD_PLAN = ([("gamma", None)] + [("A", (0,)), ("A", (1,))] +
               [("A", (2, 3)), ("A", (4, 5)), ("A", (6, 7)), ("A", (8, 9))])
SYNC_PLAN = ([("W", (g,)) for g in range(4)] +
             [("B", (8, 9)), ("B", (10, 11))])


def _pan0_schedule():
    """Greedy og0-3 matmul order under HW-measured DMA arrival times
    (queue spin-up: sync 8.8us @108GB/s, scalar 10.6us @70GB/s, gpsimd
    13.8us @73GB/s). Returns (oi, slot) and ("fill",) entries;
    correctness is enforced by Tile semaphores - this only minimizes
    issue-order stalls, and filler matmuls absorb the DMA-gated idle so
    HAM never re-throttles."""
    xt_av = [0.0] * NS
    w_av = [0.0] * 4
    t = 10.6
    for kind, payload in SCALAR_PLAN:
        t += 0.262 * len(payload) * (1e3 / 60.0)
        for s in payload:
            xt_av[s] = t
    t = 13.8
    for kind, payload in GPSIMD_PLAN:
        if kind == "gamma":
            t += 0.3
            continue
        t += 0.262 * len(payload) * (1e3 / 95.0)
        for j in payload:
            xt_av[NB + j] = t
    t = 8.8
    for kind, payload in SYNC_PLAN:
        if kind == "W":
            t += 0.524 * (1e3 / 108.0)
            w_av[payload[0]] = t
        else:
            t += 0.262 * len(payload) * (1e3 / 108.0)
            for s in payload:
                xt_av[s] = t

    nxt = [0] * 4
    seq = []
    t = 14.0
    n_fill = 0
    while any(n < NS for n in nxt):
        progressed = False
        for oi in range(4):
            if nxt[oi] >= NS or t < w_av[oi] or t < xt_av[nxt[oi]]:
                continue
            seq.append((oi, nxt[oi]))
            nxt[oi] += 1
            t += 2 * 0.216
            progressed = True
        if not progressed:
            pend = [xt_av[nxt[oi]] for oi in range(4) if nxt[oi] < NS]
            pend += [w_av[oi] for oi in range(4) if nxt[oi] == 0]
            fut = [p for p in pend if p > t]
            tn = max(t + 0.216, min(fut)) if fut else t + 0.216
            # fillers keep the PE busy only before og3 starts (its PSUM
            # bank tag doubles as the filler bank)
            if nxt[3] == 0 and n_fill < 60:
                k = min(int((tn - t) / 0.216) + 1, 60 - n_fill)
                seq += [("fill", None)] * k
                n_fill += k
            t = tn
    return seq


def build_nc(do_compile=True):
    nc = bacc.Bacc(
        "TRN2",
        target_bir_lowering=False,
        debug=False,
        enable_asserts=True,
        num_devices=N_CORES,
    )

    xt_ext = nc.declare_dram_parameter("xt", [NS, P, 2, TOK], FP8, isOutput=False)
    xs_ext = nc.declare_dram_parameter("xs", [TOK, D_IN], FP8, isOutput=False)
    # merged weights: w[g, i, :] = [wb rows | wa rows], 4KB/partition/og
    w_ext = nc.declare_dram_parameter("w", [NOG, P, WROW], FP8, isOutput=False)
    gamma_ext = nc.declare_dram_parameter("gamma", [D_OUT], F32, isOutput=False)
    rstd_ext = nc.declare_dram_parameter("rstd", [TOK], F32, isOutput=True)
    out_ext = nc.declare_dram_parameter("out", [D_OUT, TOK], BF16, isOutput=True)

    def row_bcast(ap_1d):
        return bass.AP(
            tensor=ap_1d.tensor, offset=ap_1d.offset,
            ap=[[0, P]] + list(ap_1d.ap),
        )

    with tile.TileContext(nc) as tc:
        with (
            tc.tile_pool(name="singles", bufs=1) as singles,
            tc.tile_pool(name="xspool", bufs=3) as xspool,
            tc.tile_pool(name="stats", bufs=2) as stats,
            tc.tile_pool(name="wpool", bufs=7) as wpool,
            tc.tile_pool(name="opool", bufs=18) as opool,
            tc.tile_pool(name="psum", bufs=1, space="PSUM") as psum,
        ):
            # ---- PE warmup first: issues during the DMA preamble ----
            warm_l = singles.tile([P, P], BF16)
            warm_r = singles.tile([P, TN], BF16)
            nc.vector.memset(warm_l, 0.0)
            nc.vector.memset(warm_r, 0.0)
            warm_ps = psum.tile([P, TN], F32, tag="ps0", name="warm_ps")
            for i in range(N_WARM):
                nc.tensor.matmul(
                    warm_ps, lhsT=warm_l, rhs=warm_r,
                    start=(i == 0), stop=(i == N_WARM - 1),
                )

            # ---- constants ----
            gamma_sb = singles.tile([P, NOG], F32)
            eps_sb = singles.tile([P, 1], F32)
            nc.vector.memset(eps_sb, EPS_NORM)
            rstd_all = singles.tile([P, 8], F32)
            rstd_bc = singles.tile([P, TOK], F32)

            xt_view = [None] * NS   # slot -> (tile, idx)
            w_view = {}             # og -> (tile, idx)

            def load_xt(slots, eng):
                n = len(slots)
                t = singles.tile(
                    [P, n, 2, TOK], FP8, tag=f"xt{slots[0]}",
                    name=f"xt_{slots[0]}",
                )
                eng.dma_start(
                    out=t,
                    in_=xt_ext[slots[0] : slots[0] + n].rearrange(
                        "s p two t -> p s two t"
                    ),
                )
                for j, s in enumerate(slots):
                    xt_view[s] = (t, j)

            def load_w(gs):
                n = len(gs)
                t = wpool.tile([P, n, WROW], FP8, tag="w", name=f"w_{gs[0]}")
                nc.sync.dma_start(
                    out=t,
                    in_=w_ext[gs[0] : gs[0] + n].rearrange("g p r -> p g r"),
                )
                for j, g in enumerate(gs):
                    w_view[g] = (t, j)

            # ---- startup DMAs ----
            for kind, payload in SYNC_PLAN:
                if kind == "W":
                    load_w(payload)
                else:
                    load_xt(payload, nc.sync)
            load_w((4, 5))
            load_w((6, 7))
            for kind, payload in SCALAR_PLAN:
                load_xt(payload, nc.scalar)
            for kind, payload in GPSIMD_PLAN:
                if kind == "gamma":
                    nc.gpsimd.dma_start(
                        out=gamma_sb,
                        in_=gamma_ext.ap().rearrange("(g p) -> p g", p=P),
                    )
                else:
                    load_xt(tuple(NB + j for j in payload), nc.gpsimd)

            xs_tiles = [None] * 8
            for st in range(4, 8):
                t = xspool.tile([P, D_IN], FP8, tag="xs", name=f"xs_{st}")
                nc.gpsimd.dma_start(out=t, in_=xs_ext[st * P : (st + 1) * P, :])
                xs_tiles[st] = t
            for st in range(0, 4):
                t = xspool.tile([P, D_IN], FP8, tag="xs", name=f"xs_{st}")
                nc.scalar.dma_start(out=t, in_=xs_ext[st * P : (st + 1) * P, :])
                xs_tiles[st] = t

            # ---- norm statistics for one token half (4 strips) ----
            def stats_half(h):
                for st in range(h * 4, h * 4 + 4):
                    sq_dummy = stats.tile(
                        [P, D_IN], BF16, tag="sq", name=f"sq_{st}"
                    )
                    sumsq = stats.tile([P, 1], F32, tag="ss", name=f"ss_{st}")
                    nc.scalar.activation(
                        out=sq_dummy,
                        in_=xs_tiles[st],
                        func=mybir.ActivationFunctionType.Square,
                        accum_out=sumsq,
                    )
                    rcol = rstd_all[:, st : st + 1]
                    nc.scalar.activation(
                        out=rcol,
                        in_=sumsq,
                        func=mybir.ActivationFunctionType.Sqrt,
                        bias=eps_sb,
                        scale=1.0 / D_IN,
                    )
                    nc.vector.reciprocal(out=rcol, in_=rcol)
                nc.scalar.dma_start(
                    out=rstd_ext[h * TN : (h + 1) * TN].rearrange(
                        "(s p) -> p s", p=P
                    ),
                    in_=rstd_all[:, h * 4 : h * 4 + 4],
                )
                nc.scalar.dma_start(
                    out=rstd_bc[:, h * TN : (h + 1) * TN],
                    in_=row_bcast(rstd_ext[h * TN : (h + 1) * TN]),
                )

            def slot_lhsT(g, s):
                wt, gi = w_view[g]
                if s < NB:
                    return (
                        wt[:, gi, s * P : (s + 1) * P]
                        .unsqueeze(1)
                        .broadcast_to([P, 2, P])
                    )
                off = NB * P + (s - NB) * 2 * P
                return wt[:, gi, off : off + 2 * P].rearrange(
                    "p (two m) -> p two m", two=2
                )

            def mm(ps_pair, g, s, first, last):
                lhsT = slot_lhsT(g, s)
                for th in range(TH):
                    xt_t, j = xt_view[s]
                    nc.tensor.matmul(
                        ps_pair[th],
                        lhsT=lhsT,
                        rhs=xt_t[:, j, :, th * TN : (th + 1) * TN],
                        start=first,
                        stop=last,
                        perf_mode=DR,
                    )

            deferred = []  # (g, o_tile) awaiting rstd mul + out DMA

            def og_epilogue(g, ps_pair):
                o_tile = opool.tile([P, TOK], BF16, tag="o", name=f"o_{g}")
                if g >= NOG - 4:
                    # last quad: these banks have no successors, so fold
                    # the rstd mul into the drain and skip the copy
                    for th in range(TH):
                        nc.vector.tensor_mul(
                            o_tile[:, th * TN : (th + 1) * TN], ps_pair[th],
                            rstd_bc[:, th * TN : (th + 1) * TN],
                        )
                    nc.scalar.activation(
                        out=o_tile,
                        in_=o_tile,
                        func=mybir.ActivationFunctionType.Copy,
                        scale=gamma_sb[:, g : g + 1],
                    )
                    eng = (nc.gpsimd, nc.scalar, nc.sync)[g % 3]
                    eng.dma_start(out=out_ext[g * P : (g + 1) * P, :], in_=o_tile)
                    return
                for th in range(TH):
                    nc.vector.tensor_copy(
                        o_tile[:, th * TN : (th + 1) * TN], ps_pair[th]
                    )
                nc.scalar.activation(
                    out=o_tile,
                    in_=o_tile,
                    func=mybir.ActivationFunctionType.Copy,
                    scale=gamma_sb[:, g : g + 1],
                )
                deferred.append((g, o_tile))
                if g == DEFER_OG - 1 or g >= DEFER_OG:
                    while deferred:
                        dg, dt = deferred.pop(0)
                        nc.vector.tensor_mul(dt, dt, rstd_bc)
                        eng = nc.gpsimd if dg % 2 == 0 else nc.scalar
                        eng.dma_start(
                            out=out_ext[dg * P : (dg + 1) * P, :], in_=dt
                        )

            def alloc_ps(g):
                b = (g % 4) * TH
                return [
                    psum.tile([P, TN], F32, tag=f"ps{b + th}", name=f"ps_{g}_{th}")
                    for th in range(TH)
                ]

            # ---- og 0-3: arrival-aware greedy order; filler matmuls
            # (into og3's not-yet-started bank) bridge DMA-gated idle ----
            fill_ps = psum.tile([P, TN], F32, tag="ps7", name="fill_ps")
            ps_of = {oi: alloc_ps(oi) for oi in range(4)}
            prog = [0] * 4
            for oi, s in _pan0_schedule():
                if oi == "fill":
                    nc.tensor.matmul(
                        fill_ps, lhsT=warm_l, rhs=warm_r,
                        start=True, stop=True,
                    )
                    continue
                mm(ps_of[oi], oi, s, prog[oi] == 0, prog[oi] == NS - 1)
                prog[oi] += 1
                if prog[oi] == NS:
                    og_epilogue(oi, ps_of[oi])
                    if oi == 3:
                        stats_half(1)
                        stats_half(0)

            # ---- og 4-31: og-major, everything resident/prefetched ----
            for g in range(4, NOG):
                if g % 2 == 0:
                    pre = g + 4
                    if pre < NOG and pre not in w_view:
                        load_w((pre, pre + 1))
                ps_pair = alloc_ps(g)
                for s in range(NS):
                    mm(ps_pair, g, s, s == 0, s == NS - 1)
                og_epilogue(g, ps_pair)

    if do_compile:
        nc.compile()
    return nc


_NC_CACHE = {}


def _pack_weights(w_q):
    w8 = np.asarray(w_q, dtype=np.float32).astype(E4)
    # wb[g, i, j, m] = w_q[128g + m, 128j + i]           (k = 128j + i)
    wB = w8[:, : NB * P].reshape(NOG, P, NB, P).transpose(0, 3, 2, 1)
    # wa[g, i, j, l, m] = w_q[128g + m, NB*128 + 256j + 128l + i]
    wA = w8[:, NB * P :].reshape(NOG, P, NA, 2, P).transpose(0, 4, 2, 3, 1)
    wflat = np.concatenate(
        [wB.reshape(NOG, P, NB * P), wA.reshape(NOG, P, NA * 2 * P)], axis=-1
    )
    return np.ascontiguousarray(wflat)


def _pack_x(xc_hi, xc_lo):
    xt = np.empty((NS, P, 2, TOK), dtype=E4)
    hiB = xc_hi[:, : NB * P].reshape(TOK, NB, P)
    loB = xc_lo.reshape(TOK, NB, P)
    xt[:NB, :, 0, :] = hiB.transpose(1, 2, 0)
    xt[:NB, :, 1, :] = loB.transpose(1, 2, 0)
    hiA = xc_hi[:, NB * P :].reshape(TOK, NA, 2, P)
    xt[NB:, :, :, :] = hiA.transpose(1, 3, 2, 0)
    return xt


def kernel(x, norm_weight, w_q, gamma):
    global LAST_RESULTS
    x32 = np.ascontiguousarray(np.asarray(x, dtype=np.float32)).reshape(
        TOK_TOTAL, D_IN
    )
    nw = np.asarray(norm_weight, dtype=np.float32)
    g32 = np.ascontiguousarray(np.asarray(gamma, dtype=np.float32))

    hi_raw = x32.astype(E4)       # stats source (pre norm_weight)
    if np.all(nw == 1.0):
        hi = hi_raw
    else:
        x32 = x32 * nw
        hi = x32.astype(E4)
    lo = (x32[:, : NB * P] - hi[:, : NB * P].astype(np.float32)).astype(E4)

    wflat = _pack_weights(w_q)

    if "nc" not in _NC_CACHE:
        _NC_CACHE["nc"] = build_nc()
    nc = _NC_CACHE["nc"]

    in_maps = []
    for c in range(N_CORES):
        sl = slice(c * TOK, (c + 1) * TOK)
        in_maps.append(
            {
                "xt": _pack_x(hi[sl], lo[sl]),
                "xs": np.ascontiguousarray(hi_raw[sl]),
                "w": wflat,
                "gamma": g32,
            }
        )
    res = run_bass_kernel_spmd(nc, in_maps, core_ids=list(range(N_CORES)))
    LAST_RESULTS = res
    out = np.empty((TOK_TOTAL, D_OUT), dtype=np.float32)
    for c in range(N_CORES):
        ot = np.asarray(res.results[c]["out"])  # [D_OUT, TOK] bf16
        out[c * TOK : (c + 1) * TOK] = ot.T
    return out.reshape(B, S_SEQ, D_OUT)


# revision 4
# speedup vs baseline: 1.1852x; 1.0051x over previous
"""BitLinear (RMSNorm + ternary linear) Trainium2 kernel, 8-way SPMD,
fp8 e4m3 DoubleRow TensorE path.

Math (identical to the reference up to quantization):
    rms   = sqrt(mean(x^2, axis=-1) + 1e-6)
    y     = (x @ w_q.T) / rms * gamma        (norm_weight == 1 fast path)

Precision scheme: x is quantized to fp8 e4m3 (hi); for k < NB*128 the
residual lo = e4m3(x - hi) rides the second DoubleRow lane. Ternary
weights are exact in fp8. Each DoubleRow matmul contracts 256 values
(2 fp8 lanes/cell, 2 MACs/cycle; HW-verified 216ns per
[256k x 128o x 512t] instruction), so the GEMM costs NS=23 slot-matmuls
per output tile instead of 32 bf16 matmuls: 0.72x the bf16 PE time at
~1.80% rel error (exact-input simulated + HW-verified; gate 2e-2).

Slot layout (identical for every output group):
  slots 0..NB-1   type-B: k in [128j, 128j+128), lane0=hi, lane1=lo,
                  weights stored once, lane dim stride-0 broadcast.
  slots NB..NS-1  type-A: k in [NB*128+256j, ...+256), two k-tiles.

Device pipeline per core (1024 tokens, data-parallel over 8 cores):
  - 32 output groups of 128; per og, 23 slot-matmuls x 2 token halves
    accumulate in 2 PSUM banks; og-major order so banks release (DVE
    bf16 copy) and epilogues stagger continuously; 8 bank tags give a
    4-og in-flight window. Stationary weight tiles (merged wb+wa, 4KB
    per partition per og) stream on the sync queue as 1MB og-pair DMAs.
  - Panel 0 (og 0-3) issues matmuls in a DMA-arrival-aware greedy order
    while xt (6MB) streams on scalar/gpsimd; afterwards xt is resident.
  - rstd never gates bank release: per-og epilogue is DVE copy ->
    ACT gamma (per-partition scale); the DVE rstd row-mul + out DMA are
    deferred for og 0-11 (by then the stats pipeline - ACT Square/Sqrt,
    DVE reciprocal on a t-major fp8 x copy, DRAM-bounced row broadcast -
    has certainly drained) and inline afterwards.
Output leaves transposed ([D_OUT, TOK] per core); host transposes back.
"""

import numpy as np
import ml_dtypes

import concourse.bass as bass
import concourse.tile as tile
from concourse import bacc, mybir
from concourse.bass_utils import run_bass_kernel_spmd

N_CORES = 8
B, S_SEQ, D_IN = 2, 4096, 4096
D_OUT = 4096
TOK_TOTAL = B * S_SEQ          # 8192
TOK = TOK_TOTAL // N_CORES     # 1024 tokens per core
P = 128
NB = 12                        # type-B slots (hi/lo, k < 1536)
NA = (D_IN - NB * P) // 256    # 10 type-A slots (hi/hi)
NS = NB + NA                   # 22 k slots
NOG = D_OUT // P               # 32 output groups of 128
TH = 2                         # token halves
TN = TOK // TH                 # 512 tokens per matmul
WROW = NB * P + NA * 2 * P     # 4096 weight bytes/partition/og
EPS_NORM = 1e-6
N_WARM = 14
DEFER_OG = 12                  # ogs whose rstd-mul + out DMA are deferred

F32 = mybir.dt.float32
BF16 = mybir.dt.bfloat16
FP8 = mybir.dt.float8e4
DR = mybir.MatmulPerfMode.DoubleRow
E4 = ml_dtypes.float8_e4m3

LAST_RESULTS = None

# startup DMA plans (B8-11 ride the fast sync queue between weight loads)
SCALAR_PLAN = ([("B", (s,)) for s in range(4)] +
               [("B", (4, 5)), ("B", (6, 7))])
GPSIMD_PLAN = ([("gamma", None)] + [("A", (0,)), ("A", (1,))] +
               [("A", (2, 3)), ("A", (4, 5)), ("A", (6, 7)), ("A", (8, 9))])
SYNC_PLAN = ([("W", (g,)) for g in range(4)] +
             [("B", (8, 9)), ("B", (10, 11))])


def _pan0_schedule():
    """Greedy og0-7 matmul order under HW-measured DMA arrival times
    (queue spin-up: sync 8.8us @108GB/s, scalar 10.6us @60GB/s, gpsimd
    13.8us @95GB/s). og4-7 unlock when their weights land AND their
    PSUM banks' previous owner (og-4) has fully issued, so real work -
    not fillers - covers most of the DMA-bound window. Returns
    (oi, slot) and ("fill",) entries; correctness is enforced by Tile
    semaphores - this only minimizes issue-order stalls."""
    xt_av = [0.0] * NS
    w_av = [0.0] * 8
    t = 10.6
    for kind, payload in SCALAR_PLAN:
        t += 0.262 * len(payload) * (1e3 / 60.0)
        for s in payload:
            xt_av[s] = t
    t = 13.8
    for kind, payload in GPSIMD_PLAN:
        if kind == "gamma":
            t += 0.3
            continue
        t += 0.262 * len(payload) * (1e3 / 95.0)
        for j in payload:
            xt_av[NB + j] = t
    t = 8.8
    for kind, payload in SYNC_PLAN:
        if kind == "W":
            t += 0.524 * (1e3 / 108.0)
            w_av[payload[0]] = t
        else:
            t += 0.262 * len(payload) * (1e3 / 108.0)
            for s in payload:
                xt_av[s] = t
    t += 1.048 * (1e3 / 108.0)
    w_av[4] = w_av[5] = t
    t += 1.048 * (1e3 / 108.0)
    w_av[6] = w_av[7] = t

    nxt = [0] * 8
    seq = []
    t = 14.0
    n_fill = 0
    while any(n < NS for n in nxt):
        picked = False
        for oi in range(8):
            if nxt[oi] >= NS:
                continue
            if oi >= 4 and nxt[oi - 4] < NS:
                continue  # PSUM bank generation not yet free
            if t < w_av[oi] or t < xt_av[nxt[oi]]:
                continue
            seq.append((oi, nxt[oi]))
            nxt[oi] += 1
            t += 2 * 0.216
            picked = True
            break  # depth-first: finish low ogs first to free banks
        if not picked:
            cand = []
            for oi in range(8):
                if nxt[oi] >= NS or (oi >= 4 and nxt[oi - 4] < NS):
                    continue
                cand.append(max(w_av[oi], xt_av[nxt[oi]]))
            fut = [c for c in cand if c > t]
            tn = max(t + 0.216, min(fut)) if fut else t + 0.216
            # fillers only before og3 starts (og3's bank doubles as the
            # filler bank)
            if nxt[3] == 0 and n_fill < 20:
                k = min(int((tn - t) / 0.216) + 1, 20 - n_fill)
                seq += [("fill", None)] * k
                n_fill += k
            t = tn
    return seq


def build_nc(do_compile=True):
    nc = bacc.Bacc(
        "TRN2",
        target_bir_lowering=False,
        debug=False,
        enable_asserts=True,
        num_devices=N_CORES,
    )

    xt_ext = nc.declare_dram_parameter("xt", [NS, P, 2, TOK], FP8, isOutput=False)
    xs_ext = nc.declare_dram_parameter("xs", [TOK, D_IN], FP8, isOutput=False)
    # merged weights: w[g, i, :] = [wb rows | wa rows], 4KB/partition/og
    w_ext = nc.declare_dram_parameter("w", [NOG, P, WROW], FP8, isOutput=False)
    gamma_ext = nc.declare_dram_parameter("gamma", [D_OUT], F32, isOutput=False)
    rstd_ext = nc.declare_dram_parameter("rstd", [TOK], F32, isOutput=True)
    out_ext = nc.declare_dram_parameter("out", [D_OUT, TOK], BF16, isOutput=True)

    def row_bcast(ap_1d):
        return bass.AP(
            tensor=ap_1d.tensor, offset=ap_1d.offset,
            ap=[[0, P]] + list(ap_1d.ap),
        )

    with tile.TileContext(nc) as tc:
        with (
            tc.tile_pool(name="singles", bufs=1) as singles,
            tc.tile_pool(name="xspool", bufs=3) as xspool,
            tc.tile_pool(name="stats", bufs=2) as stats,
            tc.tile_pool(name="wpool", bufs=7) as wpool,
            tc.tile_pool(name="opool", bufs=18) as opool,
            tc.tile_pool(name="psum", bufs=1, space="PSUM") as psum,
        ):
            # ---- PE warmup first: issues during the DMA preamble ----
            warm_l = singles.tile([P, P], BF16)
            warm_r = singles.tile([P, TN], BF16)
            nc.vector.memset(warm_l, 0.0)
            nc.vector.memset(warm_r, 0.0)
            warm_ps = psum.tile([P, TN], F32, tag="ps0", name="warm_ps")
            for i in range(N_WARM):
                nc.tensor.matmul(
                    warm_ps, lhsT=warm_l, rhs=warm_r,
                    start=(i == 0), stop=(i == N_WARM - 1),
                )

            # ---- constants ----
            gamma_sb = singles.tile([P, NOG], F32)
            eps_sb = singles.tile([P, 1], F32)
            nc.vector.memset(eps_sb, EPS_NORM)
            rstd_all = singles.tile([P, 8], F32)
            rstd_bc = singles.tile([P, TOK], F32)

            xt_view = [None] * NS   # slot -> (tile, idx)
            w_view = {}             # og -> (tile, idx)

            def load_xt(slots, eng):
                n = len(slots)
                t = singles.tile(
                    [P, n, 2, TOK], FP8, tag=f"xt{slots[0]}",
                    name=f"xt_{slots[0]}",
                )
                eng.dma_start(
                    out=t,
                    in_=xt_ext[slots[0] : slots[0] + n].rearrange(
                        "s p two t -> p s two t"
                    ),
                )
                for j, s in enumerate(slots):
                    xt_view[s] = (t, j)

            def load_w(gs):
                n = len(gs)
                t = wpool.tile([P, n, WROW], FP8, tag="w", name=f"w_{gs[0]}")
                nc.sync.dma_start(
                    out=t,
                    in_=w_ext[gs[0] : gs[0] + n].rearrange("g p r -> p g r"),
                )
                for j, g in enumerate(gs):
                    w_view[g] = (t, j)

            # ---- startup DMAs ----
            for kind, payload in SYNC_PLAN:
                if kind == "W":
                    load_w(payload)
                else:
                    load_xt(payload, nc.sync)
            load_w((4, 5))
            load_w((6, 7))
            load_w((8, 9))
            load_w((10, 11))
            for kind, payload in SCALAR_PLAN:
                load_xt(payload, nc.scalar)
            for kind, payload in GPSIMD_PLAN:
                if kind == "gamma":
                    nc.gpsimd.dma_start(
                        out=gamma_sb,
                        in_=gamma_ext.ap().rearrange("(g p) -> p g", p=P),
                    )
                else:
                    load_xt(tuple(NB + j for j in payload), nc.gpsimd)

            xs_tiles = [None] * 8
            for st in range(4, 8):
                t = xspool.tile([P, D_IN], FP8, tag="xs", name=f"xs_{st}")
                nc.gpsimd.dma_start(out=t, in_=xs_ext[st * P : (st + 1) * P, :])
                xs_tiles[st] = t
            for st in range(0, 4):
                t = xspool.tile([P, D_IN], FP8, tag="xs", name=f"xs_{st}")
                nc.scalar.dma_start(out=t, in_=xs_ext[st * P : (st + 1) * P, :])
                xs_tiles[st] = t

            # ---- norm statistics for one token half (4 strips) ----
            def stats_half(h):
                for st in range(h * 4, h * 4 + 4):
                    sq_dummy = stats.tile(
                        [P, D_IN], BF16, tag="sq", name=f"sq_{st}"
                    )
                    sumsq = stats.tile([P, 1], F32, tag="ss", name=f"ss_{st}")
                    nc.scalar.activation(
                        out=sq_dummy,
                        in_=xs_tiles[st],
                        func=mybir.ActivationFunctionType.Square,
                        accum_out=sumsq,
                    )
                    rcol = rstd_all[:, st : st + 1]
                    nc.scalar.activation(
                        out=rcol,
                        in_=sumsq,
                        func=mybir.ActivationFunctionType.Sqrt,
                        bias=eps_sb,
                        scale=1.0 / D_IN,
                    )
                    nc.vector.reciprocal(out=rcol, in_=rcol)
                nc.scalar.dma_start(
                    out=rstd_ext[h * TN : (h + 1) * TN].rearrange(
                        "(s p) -> p s", p=P
                    ),
                    in_=rstd_all[:, h * 4 : h * 4 + 4],
                )
                nc.scalar.dma_start(
                    out=rstd_bc[:, h * TN : (h + 1) * TN],
                    in_=row_bcast(rstd_ext[h * TN : (h + 1) * TN]),
                )

            def slot_lhsT(g, s):
                wt, gi = w_view[g]
                if s < NB:
                    return (
                        wt[:, gi, s * P : (s + 1) * P]
                        .unsqueeze(1)
                        .broadcast_to([P, 2, P])
                    )
                off = NB * P + (s - NB) * 2 * P
                return wt[:, gi, off : off + 2 * P].rearrange(
                    "p (two m) -> p two m", two=2
                )

            def mm(ps_pair, g, s, first, last):
                lhsT = slot_lhsT(g, s)
                for th in range(TH):
                    xt_t, j = xt_view[s]
                    nc.tensor.matmul(
                        ps_pair[th],
                        lhsT=lhsT,
                        rhs=xt_t[:, j, :, th * TN : (th + 1) * TN],
                        start=first,
                        stop=last,
                        perf_mode=DR,
                    )

            deferred = []  # (g, o_tile) awaiting rstd mul + out DMA

            def og_epilogue(g, ps_pair):
                o_tile = opool.tile([P, TOK], BF16, tag="o", name=f"o_{g}")
                if g >= NOG - 4:
                    # last quad: these banks have no successors, so fold
                    # the rstd mul into the drain and skip the copy
                    for th in range(TH):
                        nc.vector.tensor_mul(
                            o_tile[:, th * TN : (th + 1) * TN], ps_pair[th],
                            rstd_bc[:, th * TN : (th + 1) * TN],
                        )
                    nc.scalar.activation(
                        out=o_tile,
                        in_=o_tile,
                        func=mybir.ActivationFunctionType.Copy,
                        scale=gamma_sb[:, g : g + 1],
                    )
                    eng = (nc.gpsimd, nc.scalar, nc.sync)[g % 3]
                    eng.dma_start(out=out_ext[g * P : (g + 1) * P, :], in_=o_tile)
                    return
                for th in range(TH):
                    nc.vector.tensor_copy(
                        o_tile[:, th * TN : (th + 1) * TN], ps_pair[th]
                    )
                nc.scalar.activation(
                    out=o_tile,
                    in_=o_tile,
                    func=mybir.ActivationFunctionType.Copy,
                    scale=gamma_sb[:, g : g + 1],
                )
                deferred.append((g, o_tile))
                if g == DEFER_OG - 1 or g >= DEFER_OG:
                    while deferred:
                        dg, dt = deferred.pop(0)
                        nc.vector.tensor_mul(dt, dt, rstd_bc)
                        eng = nc.gpsimd if dg % 2 == 0 else nc.scalar
                        eng.dma_start(
                            out=out_ext[dg * P : (dg + 1) * P, :], in_=dt
                        )

            def alloc_ps(g):
                b = (g % 4) * TH
                return [
                    psum.tile([P, TN], F32, tag=f"ps{b + th}", name=f"ps_{g}_{th}")
                    for th in range(TH)
                ]

            # ---- og 0-7: arrival-aware greedy order; filler matmuls
            # (into og3's not-yet-started bank) bridge DMA-gated idle ----
            fill_ps = psum.tile([P, TN], F32, tag="ps7", name="fill_ps")
            ps_of = {}
            prog = [0] * 8
            for oi, s in _pan0_schedule():
                if oi == "fill":
                    nc.tensor.matmul(
                        fill_ps, lhsT=warm_l, rhs=warm_r,
                        start=True, stop=True,
                    )
                    continue
                if prog[oi] == 0:
                    ps_of[oi] = alloc_ps(oi)
                mm(ps_of[oi], oi, s, prog[oi] == 0, prog[oi] == NS - 1)
                prog[oi] += 1
                if prog[oi] == NS:
                    og_epilogue(oi, ps_of[oi])
                    if oi == 3:
                        stats_half(1)
                        stats_half(0)

            # ---- og 8-31: og-major, everything resident/prefetched ----
            for g in range(8, NOG):
                if g % 2 == 0:
                    pre = g + 4
                    if pre < NOG and pre not in w_view:
                        load_w((pre, pre + 1))
                ps_pair = alloc_ps(g)
                for s in range(NS):
                    mm(ps_pair, g, s, s == 0, s == NS - 1)
                og_epilogue(g, ps_pair)

    if do_compile:
        nc.compile()
    return nc


_NC_CACHE = {}


def _pack_weights(w_q):
    w8 = np.asarray(w_q, dtype=np.float32).astype(E4)
    # wb[g, i, j, m] = w_q[128g + m, 128j + i]           (k = 128j + i)
    wB = w8[:, : NB * P].reshape(NOG, P, NB, P).transpose(0, 3, 2, 1)
    # wa[g, i, j, l, m] = w_q[128g + m, NB*128 + 256j + 128l + i]
    wA = w8[:, NB * P :].reshape(NOG, P, NA, 2, P).transpose(0, 4, 2, 3, 1)
    wflat = np.concatenate(
        [wB.reshape(NOG, P, NB * P), wA.reshape(NOG, P, NA * 2 * P)], axis=-1
    )
    return np.ascontiguousarray(wflat)


def _pack_x(xc_hi, xc_lo):
    xt = np.empty((NS, P, 2, TOK), dtype=E4)
    hiB = xc_hi[:, : NB * P].reshape(TOK, NB, P)
    loB = xc_lo.reshape(TOK, NB, P)
    xt[:NB, :, 0, :] = hiB.transpose(1, 2, 0)
    xt[:NB, :, 1, :] = loB.transpose(1, 2, 0)
    hiA = xc_hi[:, NB * P :].reshape(TOK, NA, 2, P)
    xt[NB:, :, :, :] = hiA.transpose(1, 3, 2, 0)
    return xt


def kernel(x, norm_weight, w_q, gamma):
    global LAST_RESULTS
    x32 = np.ascontiguousarray(np.asarray(x, dtype=np.float32)).reshape(
        TOK_TOTAL, D_IN
    )
    nw = np.asarray(norm_weight, dtype=np.float32)
    g32 = np.ascontiguousarray(np.asarray(gamma, dtype=np.float32))

    hi_raw = x32.astype(E4)       # stats source (pre norm_weight)
    if np.all(nw == 1.0):
        hi = hi_raw
    else:
        x32 = x32 * nw
        hi = x32.astype(E4)
    lo = (x32[:, : NB * P] - hi[:, : NB * P].astype(np.float32)).astype(E4)

    wflat = _pack_weights(w_q)

    if "nc" not in _NC_CACHE:
        _NC_CACHE["nc"] = build_nc()
    nc = _NC_CACHE["nc"]

    in_maps = []
    for c in range(N_CORES):
        sl = slice(c * TOK, (c + 1) * TOK)
        in_maps.append(
            {
                "xt": _pack_x(hi[sl], lo[sl]),
                "xs": np.ascontiguousarray(hi_raw[sl]),
                "w": wflat,
                "gamma": g32,
            }
        )
    res = run_bass_kernel_spmd(nc, in_maps, core_ids=list(range(N_CORES)))
    LAST_RESULTS = res
    out = np.empty((TOK_TOTAL, D_OUT), dtype=np.float32)
    for c in range(N_CORES):
        ot = np.asarray(res.results[c]["out"])  # [D_OUT, TOK] bf16
        out[c * TOK : (c + 1) * TOK] = ot.T
    return out.reshape(B, S_SEQ, D_OUT)


# revision 5
# speedup vs baseline: 1.1912x; 1.0050x over previous
"""BitLinear (RMSNorm + ternary linear) Trainium2 kernel, 8-way SPMD,
fp8 e4m3 DoubleRow TensorE path.

Math (identical to the reference up to quantization):
    rms   = sqrt(mean(x^2, axis=-1) + 1e-6)
    y     = (x @ w_q.T) / rms * gamma        (norm_weight == 1 fast path)

Precision scheme: x is quantized to fp8 e4m3 (hi); for k < NB*128 the
residual lo = e4m3(x - hi) rides the second DoubleRow lane. Ternary
weights are exact in fp8. Each DoubleRow matmul contracts 256 values
(2 fp8 lanes/cell, 2 MACs/cycle; HW-verified 216ns per
[256k x 128o x 512t] instruction), so the GEMM costs NS=23 slot-matmuls
per output tile instead of 32 bf16 matmuls: 0.72x the bf16 PE time at
~1.80% rel error (exact-input simulated + HW-verified; gate 2e-2).

Slot layout (identical for every output group):
  slots 0..NB-1   type-B: k in [128j, 128j+128), lane0=hi, lane1=lo,
                  weights stored once, lane dim stride-0 broadcast.
  slots NB..NS-1  type-A: k in [NB*128+256j, ...+256), two k-tiles.

Device pipeline per core (1024 tokens, data-parallel over 8 cores):
  - 32 output groups of 128; per og, 23 slot-matmuls x 2 token halves
    accumulate in 2 PSUM banks; og-major order so banks release (DVE
    bf16 copy) and epilogues stagger continuously; 8 bank tags give a
    4-og in-flight window. Stationary weight tiles (merged wb+wa, 4KB
    per partition per og) stream on the sync queue as 1MB og-pair DMAs.
  - Panel 0 (og 0-3) issues matmuls in a DMA-arrival-aware greedy order
    while xt (6MB) streams on scalar/gpsimd; afterwards xt is resident.
  - rstd never gates bank release: per-og epilogue is DVE copy ->
    ACT gamma (per-partition scale); the DVE rstd row-mul + out DMA are
    deferred for og 0-11 (by then the stats pipeline - ACT Square/Sqrt,
    DVE reciprocal on a t-major fp8 x copy, DRAM-bounced row broadcast -
    has certainly drained) and inline afterwards.
Output leaves transposed ([D_OUT, TOK] per core); host transposes back.
"""

import numpy as np
import ml_dtypes

import concourse.bass as bass
import concourse.tile as tile
from concourse import bacc, mybir
from concourse.bass_utils import run_bass_kernel_spmd

N_CORES = 8
B, S_SEQ, D_IN = 2, 4096, 4096
D_OUT = 4096
TOK_TOTAL = B * S_SEQ          # 8192
TOK = TOK_TOTAL // N_CORES     # 1024 tokens per core
P = 128
NB = 12                        # type-B slots (hi/lo, k < 1536)
NA = (D_IN - NB * P) // 256    # 10 type-A slots (hi/hi)
NS = NB + NA                   # 22 k slots
NOG = D_OUT // P               # 32 output groups of 128
TH = 2                         # token halves
TN = TOK // TH                 # 512 tokens per matmul
WROW = NB * P + NA * 2 * P     # 4096 weight bytes/partition/og
EPS_NORM = 1e-6
N_WARM = 14
DEFER_OG = 12                  # ogs whose rstd-mul + out DMA are deferred

F32 = mybir.dt.float32
BF16 = mybir.dt.bfloat16
FP8 = mybir.dt.float8e4
DR = mybir.MatmulPerfMode.DoubleRow
E4 = ml_dtypes.float8_e4m3

LAST_RESULTS = None

# startup DMA plans (B8-11 ride the fast sync queue between weight loads)
SCALAR_PLAN = ([("B", (s,)) for s in range(4)] +
               [("B", (4, 5)), ("B", (6, 7))])
GPSIMD_PLAN = ([("gamma", None)] + [("A", (0,)), ("A", (1,))] +
               [("A", (2, 3)), ("A", (4, 5)), ("A", (6, 7)), ("A", (8, 9))])
SYNC_PLAN = ([("W", (g,)) for g in range(4)] +
             [("B", (8, 9)), ("B", (10, 11))])


def _pan0_schedule():
    """Greedy og0-7 matmul order under HW-measured DMA arrival times
    (queue spin-up: sync 8.8us @108GB/s, scalar 10.6us @60GB/s, gpsimd
    13.8us @95GB/s). og4-7 unlock when their weights land AND their
    PSUM banks' previous owner (og-4) has fully issued, so real work -
    not fillers - covers most of the DMA-bound window. Returns
    (oi, slot) and ("fill",) entries; correctness is enforced by Tile
    semaphores - this only minimizes issue-order stalls."""
    xt_av = [0.0] * NS
    w_av = [0.0] * 8
    t = 10.6
    for kind, payload in SCALAR_PLAN:
        t += 0.262 * len(payload) * (1e3 / 55.0)
        for s in payload:
            xt_av[s] = t
    t = 14.5
    for kind, payload in GPSIMD_PLAN:
        if kind == "gamma":
            t += 0.3
            continue
        t += 0.262 * len(payload) * (1e3 / 85.0)
        for j in payload:
            xt_av[NB + j] = t
    t = 8.8
    for kind, payload in SYNC_PLAN:
        if kind == "W":
            t += 0.524 * (1e3 / 100.0)
            w_av[payload[0]] = t
        else:
            t += 0.262 * len(payload) * (1e3 / 100.0)
            for s in payload:
                xt_av[s] = t
    t += 1.048 * (1e3 / 100.0)
    w_av[4] = w_av[5] = t
    t += 1.048 * (1e3 / 100.0)
    w_av[6] = w_av[7] = t

    nxt = [0] * 8
    seq = []
    t = 14.0
    n_fill = 0
    n_fill2 = 0
    while any(n < NS for n in nxt):
        picked = False
        for oi in range(8):
            if nxt[oi] >= NS:
                continue
            if oi >= 4 and nxt[oi - 4] < NS:
                continue  # PSUM bank generation not yet free
            if t < w_av[oi] or t < xt_av[nxt[oi]]:
                continue
            seq.append((oi, nxt[oi]))
            nxt[oi] += 1
            t += 2 * 0.216
            picked = True
            break  # depth-first: finish low ogs first to free banks
        if not picked:
            cand = []
            for oi in range(8):
                if nxt[oi] >= NS or (oi >= 4 and nxt[oi - 4] < NS):
                    continue
                cand.append(max(w_av[oi], xt_av[nxt[oi]]))
            fut = [c for c in cand if c > t]
            tn = max(t + 0.216, min(fut)) if fut else t + 0.216
            # fillers keep HAM warm through DMA waits: phase-1 fillers
            # use og3's not-yet-started bank; phase-2 fillers reuse
            # og0's freed bank before og4 claims it
            if nxt[3] == 0 and n_fill < 28:
                k = min(int((tn - t) / 0.216) + 1, 28 - n_fill)
                seq += [("fill", None)] * k
                n_fill += k
            elif nxt[0] >= NS and nxt[4] == 0 and n_fill2 < 28:
                k = min(int((tn - t) / 0.216) + 1, 28 - n_fill2)
                seq += [("fill2", None)] * k
                n_fill2 += k
            t = tn
    return seq


def build_nc(do_compile=True):
    nc = bacc.Bacc(
        "TRN2",
        target_bir_lowering=False,
        debug=False,
        enable_asserts=True,
        num_devices=N_CORES,
    )

    xt_ext = nc.declare_dram_parameter("xt", [NS, P, 2, TOK], FP8, isOutput=False)
    xs_ext = nc.declare_dram_parameter("xs", [TOK, D_IN], FP8, isOutput=False)
    # merged weights: w[g, i, :] = [wb rows | wa rows], 4KB/partition/og
    w_ext = nc.declare_dram_parameter("w", [NOG, P, WROW], FP8, isOutput=False)
    gamma_ext = nc.declare_dram_parameter("gamma", [D_OUT], F32, isOutput=False)
    rstd_ext = nc.declare_dram_parameter("rstd", [TOK], F32, isOutput=True)
    out_ext = nc.declare_dram_parameter("out", [D_OUT, TOK], BF16, isOutput=True)

    def row_bcast(ap_1d):
        return bass.AP(
            tensor=ap_1d.tensor, offset=ap_1d.offset,
            ap=[[0, P]] + list(ap_1d.ap),
        )

    with tile.TileContext(nc) as tc:
        with (
            tc.tile_pool(name="singles", bufs=1) as singles,
            tc.tile_pool(name="xspool", bufs=3) as xspool,
            tc.tile_pool(name="stats", bufs=2) as stats,
            tc.tile_pool(name="wpool", bufs=7) as wpool,
            tc.tile_pool(name="opool", bufs=18) as opool,
            tc.tile_pool(name="psum", bufs=1, space="PSUM") as psum,
        ):
            # ---- PE warmup first: issues during the DMA preamble ----
            warm_l = singles.tile([P, P], BF16)
            warm_r = singles.tile([P, TN], BF16)
            nc.vector.memset(warm_l, 0.0)
            nc.vector.memset(warm_r, 0.0)
            warm_ps = psum.tile([P, TN], F32, tag="ps0", name="warm_ps")
            for i in range(N_WARM):
                nc.tensor.matmul(
                    warm_ps, lhsT=warm_l, rhs=warm_r,
                    start=(i == 0), stop=(i == N_WARM - 1),
                )

            # ---- constants ----
            gamma_sb = singles.tile([P, NOG], F32)
            eps_sb = singles.tile([P, 1], F32)
            nc.vector.memset(eps_sb, EPS_NORM)
            rstd_all = singles.tile([P, 8], F32)
            rstd_bc = singles.tile([P, TOK], F32)

            xt_view = [None] * NS   # slot -> (tile, idx)
            w_view = {}             # og -> (tile, idx)

            def load_xt(slots, eng):
                n = len(slots)
                t = singles.tile(
                    [P, n, 2, TOK], FP8, tag=f"xt{slots[0]}",
                    name=f"xt_{slots[0]}",
                )
                eng.dma_start(
                    out=t,
                    in_=xt_ext[slots[0] : slots[0] + n].rearrange(
                        "s p two t -> p s two t"
                    ),
                )
                for j, s in enumerate(slots):
                    xt_view[s] = (t, j)

            def load_w(gs):
                n = len(gs)
                t = wpool.tile([P, n, WROW], FP8, tag="w", name=f"w_{gs[0]}")
                nc.sync.dma_start(
                    out=t,
                    in_=w_ext[gs[0] : gs[0] + n].rearrange("g p r -> p g r"),
                )
                for j, g in enumerate(gs):
                    w_view[g] = (t, j)

            # ---- startup DMAs ----
            for kind, payload in SYNC_PLAN:
                if kind == "W":
                    load_w(payload)
                else:
                    load_xt(payload, nc.sync)
            load_w((4, 5))
            load_w((6, 7))
            load_w((8, 9))
            load_w((10, 11))
            for kind, payload in SCALAR_PLAN:
                load_xt(payload, nc.scalar)
            for kind, payload in GPSIMD_PLAN:
                if kind == "gamma":
                    nc.gpsimd.dma_start(
                        out=gamma_sb,
                        in_=gamma_ext.ap().rearrange("(g p) -> p g", p=P),
                    )
                else:
                    load_xt(tuple(NB + j for j in payload), nc.gpsimd)

            xs_tiles = [None] * 8
            for st in range(4, 8):
                t = xspool.tile([P, D_IN], FP8, tag="xs", name=f"xs_{st}")
                nc.gpsimd.dma_start(out=t, in_=xs_ext[st * P : (st + 1) * P, :])
                xs_tiles[st] = t
            for st in range(0, 4):
                t = xspool.tile([P, D_IN], FP8, tag="xs", name=f"xs_{st}")
                nc.scalar.dma_start(out=t, in_=xs_ext[st * P : (st + 1) * P, :])
                xs_tiles[st] = t

            # ---- norm statistics for one token half (4 strips) ----
            def stats_half(h):
                for st in range(h * 4, h * 4 + 4):
                    sq_dummy = stats.tile(
                        [P, D_IN], BF16, tag="sq", name=f"sq_{st}"
                    )
                    sumsq = stats.tile([P, 1], F32, tag="ss", name=f"ss_{st}")
                    nc.scalar.activation(
                        out=sq_dummy,
                        in_=xs_tiles[st],
                        func=mybir.ActivationFunctionType.Square,
                        accum_out=sumsq,
                    )
                    rcol = rstd_all[:, st : st + 1]
                    nc.scalar.activation(
                        out=rcol,
                        in_=sumsq,
                        func=mybir.ActivationFunctionType.Sqrt,
                        bias=eps_sb,
                        scale=1.0 / D_IN,
                    )
                    nc.vector.reciprocal(out=rcol, in_=rcol)
                nc.scalar.dma_start(
                    out=rstd_ext[h * TN : (h + 1) * TN].rearrange(
                        "(s p) -> p s", p=P
                    ),
                    in_=rstd_all[:, h * 4 : h * 4 + 4],
                )
                nc.scalar.dma_start(
                    out=rstd_bc[:, h * TN : (h + 1) * TN],
                    in_=row_bcast(rstd_ext[h * TN : (h + 1) * TN]),
                )

            def slot_lhsT(g, s):
                wt, gi = w_view[g]
                if s < NB:
                    return (
                        wt[:, gi, s * P : (s + 1) * P]
                        .unsqueeze(1)
                        .broadcast_to([P, 2, P])
                    )
                off = NB * P + (s - NB) * 2 * P
                return wt[:, gi, off : off + 2 * P].rearrange(
                    "p (two m) -> p two m", two=2
                )

            def mm(ps_pair, g, s, first, last):
                lhsT = slot_lhsT(g, s)
                for th in range(TH):
                    xt_t, j = xt_view[s]
                    nc.tensor.matmul(
                        ps_pair[th],
                        lhsT=lhsT,
                        rhs=xt_t[:, j, :, th * TN : (th + 1) * TN],
                        start=first,
                        stop=last,
                        perf_mode=DR,
                    )

            deferred = []  # (g, o_tile) awaiting rstd mul + out DMA

            def og_epilogue(g, ps_pair):
                o_tile = opool.tile([P, TOK], BF16, tag="o", name=f"o_{g}")
                if g >= NOG - 4:
                    # last quad: these banks have no successors, so fold
                    # the rstd mul into the drain and skip the copy
                    for th in range(TH):
                        nc.vector.tensor_mul(
                            o_tile[:, th * TN : (th + 1) * TN], ps_pair[th],
                            rstd_bc[:, th * TN : (th + 1) * TN],
                        )
                    nc.scalar.activation(
                        out=o_tile,
                        in_=o_tile,
                        func=mybir.ActivationFunctionType.Copy,
                        scale=gamma_sb[:, g : g + 1],
                    )
                    eng = (nc.gpsimd, nc.scalar, nc.sync)[g % 3]
                    eng.dma_start(out=out_ext[g * P : (g + 1) * P, :], in_=o_tile)
                    return
                for th in range(TH):
                    nc.vector.tensor_copy(
                        o_tile[:, th * TN : (th + 1) * TN], ps_pair[th]
                    )
                nc.scalar.activation(
                    out=o_tile,
                    in_=o_tile,
                    func=mybir.ActivationFunctionType.Copy,
                    scale=gamma_sb[:, g : g + 1],
                )
                deferred.append((g, o_tile))
                if g == DEFER_OG - 1 or g >= DEFER_OG:
                    while deferred:
                        dg, dt = deferred.pop(0)
                        nc.vector.tensor_mul(dt, dt, rstd_bc)
                        eng = nc.gpsimd if dg % 2 == 0 else nc.scalar
                        eng.dma_start(
                            out=out_ext[dg * P : (dg + 1) * P, :], in_=dt
                        )

            def alloc_ps(g):
                b = (g % 4) * TH
                return [
                    psum.tile([P, TN], F32, tag=f"ps{b + th}", name=f"ps_{g}_{th}")
                    for th in range(TH)
                ]

            # ---- og 0-7: arrival-aware greedy order; filler matmuls
            # (into og3's not-yet-started bank) bridge DMA-gated idle ----
            fill_ps = psum.tile([P, TN], F32, tag="ps7", name="fill_ps")
            fill2_ps = None
            ps_of = {}
            prog = [0] * 8
            for oi, s in _pan0_schedule():
                if oi == "fill" or oi == "fill2":
                    if oi == "fill2" and fill2_ps is None:
                        # og0's bank is free until og4 claims it; phase-2
                        # fillers borrow it (first use is model-guaranteed
                        # to sit between og0's last mm and og4's first)
                        fill2_ps = psum.tile(
                            [P, TN], F32, tag="ps0", name="fill2_ps"
                        )
                    tgt = fill_ps if oi == "fill" else fill2_ps
                    nc.tensor.matmul(
                        tgt, lhsT=warm_l, rhs=warm_r,
                        start=True, stop=True,
                    )
                    continue
                if prog[oi] == 0:
                    ps_of[oi] = alloc_ps(oi)
                mm(ps_of[oi], oi, s, prog[oi] == 0, prog[oi] == NS - 1)
                prog[oi] += 1
                if prog[oi] == NS:
                    og_epilogue(oi, ps_of[oi])
                    if oi == 3:
                        stats_half(1)
                        stats_half(0)

            # ---- og 8-31: og-major, everything resident/prefetched ----
            for g in range(8, NOG):
                if g % 2 == 0:
                    pre = g + 4
                    if pre < NOG and pre not in w_view:
                        load_w((pre, pre + 1))
                ps_pair = alloc_ps(g)
                for s in range(NS):
                    mm(ps_pair, g, s, s == 0, s == NS - 1)
                og_epilogue(g, ps_pair)

    if do_compile:
        nc.compile()
    return nc


_NC_CACHE = {}


def _pack_weights(w_q):
    w8 = np.asarray(w_q, dtype=np.float32).astype(E4)
    # wb[g, i, j, m] = w_q[128g + m, 128j + i]           (k = 128j + i)
    wB = w8[:, : NB * P].reshape(NOG, P, NB, P).transpose(0, 3, 2, 1)
    # wa[g, i, j, l, m] = w_q[128g + m, NB*128 + 256j + 128l + i]
    wA = w8[:, NB * P :].reshape(NOG, P, NA, 2, P).transpose(0, 4, 2, 3, 1)
    wflat = np.concatenate(
        [wB.reshape(NOG, P, NB * P), wA.reshape(NOG, P, NA * 2 * P)], axis=-1
    )
    return np.ascontiguousarray(wflat)


def _pack_x(xc_hi, xc_lo):
    xt = np.empty((NS, P, 2, TOK), dtype=E4)
    hiB = xc_hi[:, : NB * P].reshape(TOK, NB, P)
    loB = xc_lo.reshape(TOK, NB, P)
    xt[:NB, :, 0, :] = hiB.transpose(1, 2, 0)
    xt[:NB, :, 1, :] = loB.transpose(1, 2, 0)
    hiA = xc_hi[:, NB * P :].reshape(TOK, NA, 2, P)
    xt[NB:, :, :, :] = hiA.transpose(1, 3, 2, 0)
    return xt


def kernel(x, norm_weight, w_q, gamma):
    global LAST_RESULTS
    x32 = np.ascontiguousarray(np.asarray(x, dtype=np.float32)).reshape(
        TOK_TOTAL, D_IN
    )
    nw = np.asarray(norm_weight, dtype=np.float32)
    g32 = np.ascontiguousarray(np.asarray(gamma, dtype=np.float32))

    hi_raw = x32.astype(E4)       # stats source (pre norm_weight)
    if np.all(nw == 1.0):
        hi = hi_raw
    else:
        x32 = x32 * nw
        hi = x32.astype(E4)
    lo = (x32[:, : NB * P] - hi[:, : NB * P].astype(np.float32)).astype(E4)

    wflat = _pack_weights(w_q)

    if "nc" not in _NC_CACHE:
        _NC_CACHE["nc"] = build_nc()
    nc = _NC_CACHE["nc"]

    in_maps = []
    for c in range(N_CORES):
        sl = slice(c * TOK, (c + 1) * TOK)
        in_maps.append(
            {
                "xt": _pack_x(hi[sl], lo[sl]),
                "xs": np.ascontiguousarray(hi_raw[sl]),
                "w": wflat,
                "gamma": g32,
            }
        )
    res = run_bass_kernel_spmd(nc, in_maps, core_ids=list(range(N_CORES)))
    LAST_RESULTS = res
    out = np.empty((TOK_TOTAL, D_OUT), dtype=np.float32)
    for c in range(N_CORES):
        ot = np.asarray(res.results[c]["out"])  # [D_OUT, TOK] bf16
        out[c * TOK : (c + 1) * TOK] = ot.T
    return out.reshape(B, S_SEQ, D_OUT)
